# revision 27
# baseline (speedup 1.0000x reference)
"""Distributed Trainium2 kernel for GQA attention (nn_Attention_76845554860188).

B=1, S=2048, D=1024, NH=16, NKV=4, HD=64, causal, RoPE, 8 NeuronCores.

Sharding: tensor-parallel over heads. Core c owns q-heads {2c, 2c+1} and their
shared GQA kv-head c//2. Each core projects Q/K/V for all 2048 positions and
runs causal flash-style attention for its 2 heads. The attention outputs are
exchanged with two AllToAlls (seq rows assigned round-robin mod 8, global row
= 8*r + c, so every exchange carries rows for every core); each core applies
the full output projection to its 256 assigned rows. Host reassembles the
row interleave (assemble_out).

Performance structure:
  * seq-chunk software pipeline: projection / RoPE / attention emitted per
    512-column chunk; per-chunk SBUF tiles so dependency tracking stays tight.
  * scores for the two heads run concurrently on the PE via row-tiling
    (K=64 each): krot duplicated at partitions 64..127 -> tile_position (64,0).
  * flash ordering: score block b+1 streams while ACT exps block b; PV(b)
    follows. No S^2 probability matrix is materialized.
  * PSUM->SBUF copies and causal masks run on DVE, broadcasts/memsets on
    gpsimd, keeping ACT free for the exp stream (the attention-phase
    co-bottleneck); the normalize chain is head-interleaved and each chunk's
    stage slice is DMA'd to the send buffer as soon as it is normalized.
  * asymmetric exchange: AllToAll #1 carries chunks 0-2 (rows 0-191) and
    fires after chunk 2; its 192 output rows are projected (pieces a+b)
    while chunk-3's normalize/send/AllToAll #2 run on DVE/gpsimd/CC. Only
    the 128KB AllToAll #2 plus a 64-row O-proj piece remain serial.
  * PE warm-up matmuls (on the first-arriving weight tile) ramp HAM during
    the xT DMA fill; xT chunk 0 is the first large DMA issued, per-chunk
    contiguous in DRAM.
"""

import sys

sys.path.insert(0, "/opt/trn_rl_repo")

import numpy as np
import ml_dtypes

import concourse.bass as bass
import concourse.mybir as mybir
import concourse.tile as tile
from concourse import bacc
from concourse.bass_utils import run_bass_kernel_spmd

BF16 = mybir.dt.bfloat16
F32 = mybir.dt.float32

B, S, D = 1, 2048, 1024
NH, NKV, HD = 16, 4, 64
NC_CORES = 8
HPC = NH // NC_CORES  # q heads per core = 2
SC = S // NC_CORES  # seq rows per core = 256
NDC = D // 128  # d chunks = 8
NSB = S // 128  # 128-wide seq blocks = 16
NCH = S // 512  # 512-wide seq chunks = 4
HALF = HD // 2  # 32

np_bf16 = ml_dtypes.bfloat16


def build_graph(taps=False):
    nc = bacc.Bacc(
        "TRN2", target_bir_lowering=False, debug=False, num_devices=NC_CORES
    )

    xT_e = nc.dram_tensor("xT", [128, NCH, NDC, 512], BF16, kind="ExternalInput")
    wq_e = nc.dram_tensor("wq", [128, NDC, HPC * HD], BF16, kind="ExternalInput")
    wkv_e = nc.dram_tensor("wkv", [128, NDC, 2 * HD], BF16, kind="ExternalInput")
    wo_e = nc.dram_tensor("wo", [128, NDC, D], BF16, kind="ExternalInput")
    c2_e = nc.dram_tensor("c2", [128, S], BF16, kind="ExternalInput")
    s2_e = nc.dram_tensor("s2", [128, S], BF16, kind="ExternalInput")
    ppm_e = nc.dram_tensor("ppm", [128, 128], BF16, kind="ExternalInput")
    idm_e = nc.dram_tensor("idm", [128, 128], BF16, kind="ExternalInput")
    tri_e = nc.dram_tensor("tri", [128, 128], BF16, kind="ExternalInput")
    out_e = nc.dram_tensor("out", [SC, D], F32, kind="ExternalOutput")

    send_d = [
        nc.dram_tensor("a2a_send0", [NC_CORES, 128, 192], BF16),
        nc.dram_tensor("a2a_send1", [NC_CORES, 128, 64], BF16),
    ]
    recv_d = [
        nc.dram_tensor("a2a_recv0", [NC_CORES, 128, 192], BF16),
        nc.dram_tensor("a2a_recv1", [NC_CORES, 128, 64], BF16),
    ]
    wup_s = nc.dram_tensor("wup_s", [NC_CORES, 1, 8], BF16)
    wup_r = nc.dram_tensor("wup_r", [NC_CORES, 1, 8], BF16)

    with tile.TileContext(nc) as tc:
        _body(nc, tc, xT_e, wq_e, wkv_e, wo_e, c2_e, s2_e, ppm_e, idm_e, tri_e,
              out_e, send_d, recv_d, wup_s, wup_r)

    nc.compile()
    return nc


def _body(nc, tc, xT_e, wq_e, wkv_e, wo_e, c2_e, s2_e, ppm_e, idm_e, tri_e,
          out_e, send_d, recv_d, wup_s, wup_r):
    from contextlib import ExitStack

    ctx = ExitStack()
    with ctx:
        consts = ctx.enter_context(tc.tile_pool(name="consts", bufs=1))
        work = ctx.enter_context(tc.tile_pool(name="work", bufs=1))
        ptp = ctx.enter_context(tc.tile_pool(name="pt", bufs=6))
        # PSUM: 8 banks total -> proj x2 + rope x1 + st x3 + ot x2
        projp_cm = tc.tile_pool(name="projp", bufs=2, space="PSUM")
        projp = projp_cm.__enter__()
        ropep_cm = tc.tile_pool(name="ropep", bufs=1, space="PSUM")
        ropep = ropep_cm.__enter__()
        stp_cm = tc.tile_pool(name="stp", bufs=3, space="PSUM")
        stp = stp_cm.__enter__()
        otp_cm = tc.tile_pool(name="otp", bufs=2, space="PSUM")
        otp = otp_cm.__enter__()

        # ---- warmup collective, first in collective program order ----
        wup_sb = consts.tile([NC_CORES, 1, 8], BF16, tag="wup")
        nc.gpsimd.memset(wup_sb[:], 0.0)
        nc.scalar.dma_start(out=wup_s.ap(), in_=wup_sb[:])
        nc.gpsimd.collective_compute(
            "AllToAll",
            mybir.AluOpType.bypass,
            replica_groups=[list(range(NC_CORES))],
            ins=[wup_s.ap().opt()],
            outs=[wup_r.ap().opt()],
        )

        # ---- DMA plan ----
        # sync queue:   xT c0, xT c1        (the critical-path inputs)
        # gpsimd queue: xT c2, xT c3, wo    (wo issued later, see below)
        # scalar queue: wq, wkv, consts, c2, s2 (small, never blocks xT)
        xT_sb = consts.tile([128, NDC, S], BF16, tag="xT")

        def load_xt(n, eng):
            sl = slice(512 * n, 512 * (n + 1))
            eng.dma_start(out=xT_sb[:, :, sl], in_=xT_e[:, n])

        load_xt(0, nc.sync)
        load_xt(1, nc.sync)
        # chunk-0's rope tables land before the bulk so rope c0 isn't
        # gated on the full 1MB of cos/sin tables
        wq_sb = consts.tile([128, NDC, HPC * HD], BF16, tag="wq")
        nc.scalar.dma_start(out=wq_sb[:], in_=wq_e.ap())
        c2_sb = consts.tile([128, S], BF16, tag="c2")
        nc.scalar.dma_start(out=c2_sb[:, 0:512], in_=c2_e[:, 0:512])
        s2_sb = consts.tile([128, S], BF16, tag="s2")
        nc.scalar.dma_start(out=s2_sb[:, 0:512], in_=s2_e[:, 0:512])
        idm_sb = consts.tile([128, 128], BF16, tag="idm")
        nc.scalar.dma_start(out=idm_sb[:], in_=idm_e[:, :])
        ppm_sb = consts.tile([128, 128], BF16, tag="ppm")
        nc.scalar.dma_start(out=ppm_sb[:], in_=ppm_e[:, :])
        wkv_sb = consts.tile([128, NDC, 2 * HD], BF16, tag="wkv")
        nc.scalar.dma_start(out=wkv_sb[:], in_=wkv_e.ap())
        tri_sb = consts.tile([128, 128], BF16, tag="tri")
        nc.scalar.dma_start(out=tri_sb[:], in_=tri_e[:, :])
        nc.scalar.dma_start(out=c2_sb[:, 512:S], in_=c2_e[:, 512:S])
        nc.scalar.dma_start(out=s2_sb[:, 512:S], in_=s2_e[:, 512:S])
        wo_sb = consts.tile([128, NDC, D], BF16, tag="wo")

        # ---- PE warm-up on the first weight tile (dep: wq DMA only) ----
        warm_ps = projp.tile([128, 512], F32, tag="proj")
        for r in range(16):
            nc.tensor.matmul(
                warm_ps[:, 0:128],
                lhsT=wq_sb[:, 0, :],
                rhs=wq_sb[:, 0, :],
                start=True,
                stop=True,
            )

        # ---- per-chunk persistent SBUF tiles (tight dependency tracking) ----
        qrot = [work.tile([128, 512], BF16, tag=f"qrot{n}", name=f"qrot{n}") for n in range(NCH)]
        krot2 = [work.tile([128, 512], BF16, tag=f"krot2{n}", name=f"krot2{n}") for n in range(NCH)]
        vTt = [work.tile([64, 512], BF16, tag=f"vT{n}", name=f"vT{n}") for n in range(NCH)]
        vext = [work.tile([128, 4, HD + 1], BF16, tag=f"vext{n}", name=f"vext{n}") for n in range(NCH)]
        for n in range(NCH):
            nc.gpsimd.memset(vext[n][:, :, HD : HD + 1], 1.0)
        # stage buffers per exchange half: [p, dest j, row r] (col s = 8r + j)
        stg = [
            work.tile([128, NC_CORES, 192], BF16, tag="stg0", name="stg0"),
            work.tile([128, NC_CORES, 64], BF16, tag="stg1", name="stg1"),
        ]

        scale = 1.0 / np.sqrt(HD)

        def proj_chunk(n):
            sl = slice(512 * n, 512 * (n + 1))
            q_ps = projp.tile([128, 512], F32, tag="proj", name=f"q{n}")
            for i in range(NDC):
                nc.tensor.matmul(
                    q_ps[:], lhsT=wq_sb[:, i, :], rhs=xT_sb[:, i, sl],
                    start=(i == 0), stop=(i == NDC - 1),
                )
            kv_ps = projp.tile([128, 512], F32, tag="proj", name=f"kv{n}")
            for i in range(NDC):
                nc.tensor.matmul(
                    kv_ps[:], lhsT=wkv_sb[:, i, :], rhs=xT_sb[:, i, sl],
                    start=(i == 0), stop=(i == NDC - 1),
                )
            return q_ps, kv_ps

        rope_tmp = tc.tile_pool(name="ropet", bufs=2)
        ropet = rope_tmp.__enter__()

        def rope_chunk(n, q_ps, kv_ps):
            sl = slice(512 * n, 512 * (n + 1))
            qc = ropet.tile([128, 512], BF16, tag="qc", name=f"qc{n}")
            nc.vector.tensor_tensor(
                out=qc[:], in0=q_ps[:], in1=c2_sb[:, sl], op=mybir.AluOpType.mult
            )
            qs = ropet.tile([128, 512], BF16, tag="qs", name=f"qs{n}")
            nc.vector.tensor_tensor(
                out=qs[:], in0=q_ps[:], in1=s2_sb[:, sl], op=mybir.AluOpType.mult
            )
            qrot_ps = ropep.tile([128, 512], F32, tag="rope", name=f"qr{n}")
            nc.tensor.matmul(
                qrot_ps[:], lhsT=ppm_sb[:], rhs=qs[:], start=True, stop=False
            )
            nc.tensor.matmul(
                qrot_ps[:], lhsT=idm_sb[:], rhs=qc[:], start=False, stop=True
            )
            nc.vector.tensor_copy(out=qrot[n][:], in_=qrot_ps[:])

            kc = ropet.tile([64, 512], BF16, tag="kc", name=f"kc{n}")
            nc.vector.tensor_tensor(
                out=kc[:], in0=kv_ps[0:64, :], in1=c2_sb[0:64, sl],
                op=mybir.AluOpType.mult,
            )
            ks = ropet.tile([64, 512], BF16, tag="ks", name=f"ks{n}")
            nc.vector.tensor_tensor(
                out=ks[:], in0=kv_ps[0:64, :], in1=s2_sb[0:64, sl],
                op=mybir.AluOpType.mult,
            )
            nc.vector.tensor_copy(out=vTt[n][:], in_=kv_ps[64:128, :])
            krot_ps = ropep.tile([64, 512], F32, tag="rope", name=f"kr{n}")
            nc.tensor.matmul(
                krot_ps[:], lhsT=ppm_sb[0:64, 0:64], rhs=ks[:], start=True, stop=False
            )
            nc.tensor.matmul(
                krot_ps[:], lhsT=idm_sb[0:64, 0:64], rhs=kc[:], start=False, stop=True
            )
            # duplicate krot to partitions 64..127 so head-1 score matmuls
            # run on PE row-groups 2-3 concurrently with head 0
            nc.vector.tensor_copy(out=krot2[n][0:64, :], in_=krot_ps[:])
            nc.vector.tensor_copy(out=krot2[n][64:128, :], in_=krot_ps[:])

            vt_ps = stp.tile([128, 4, HD], BF16, tag="st", name=f"vt{n}")
            for j in range(4):
                nc.tensor.transpose(
                    vt_ps[:, j, :],
                    vTt[n][:, 128 * j : 128 * (j + 1)],
                    idm_sb[0:64, 0:64],
                )
            nc.vector.tensor_copy(out=vext[n][:, :, 0:HD], in_=vt_ps[:])

        def attention_chunk(k, mid_cb=None, mid_b=None):
            nblk = 4 * k + 4
            ot = [
                otp.tile([HD + 1, 512], F32, tag="ot", name=f"ot{h}_{k}")
                for h in range(HPC)
            ]
            pend = []

            def emit_pv(b, pt, qoff, w):
                for h in range(HPC):
                    nc.tensor.matmul(
                        ot[h][:, qoff : qoff + w],
                        lhsT=vext[b // 4][:, b % 4, :],
                        rhs=pt[h][:, 0:w],
                        start=(b == 0),
                        stop=(b == nblk - 1),
                    )

            for b in range(nblk):
                qoff = max(0, 128 * b - 512 * k)
                w = 512 - qoff
                q0 = qoff  # column offset within chunk k's qrot tile
                st0 = stp.tile([128, 512], F32, tag="st", name=f"st0_{k}_{b}")
                st1 = stp.tile([128, 512], F32, tag="st", name=f"st1_{k}_{b}")
                kb = slice(128 * (b % 4), 128 * (b % 4) + 128)
                kt = krot2[b // 4]
                nc.tensor.matmul(
                    st0[:, 0:w], lhsT=kt[0:64, kb],
                    rhs=qrot[k][0:64, q0 : q0 + w], start=True, stop=True,
                )
                nc.tensor.matmul(
                    st1[:, 0:w], lhsT=kt[64:128, kb],
                    rhs=qrot[k][64:128, q0 : q0 + w], start=True, stop=True,
                )
                if pend:
                    emit_pv(*pend.pop())
                pt = [
                    ptp.tile([128, 512], BF16, tag="pt", name=f"pt{h}_{k}_{b}")
                    for h in range(HPC)
                ]
                for h, st in ((0, st0), (1, st1)):
                    nc.scalar.activation(
                        out=pt[h][:, 0:w], in_=st[:, 0:w],
                        func=mybir.ActivationFunctionType.Exp, scale=scale,
                    )
                if 128 * b >= 512 * k:
                    # diagonal block: mask its first 128 cols (kpos > q -> 0)
                    for h in range(HPC):
                        nc.vector.tensor_tensor(
                            out=pt[h][:, 0:128], in0=pt[h][:, 0:128],
                            in1=tri_sb[:], op=mybir.AluOpType.mult,
                        )
                pend.append((b, pt, qoff, w))
                if mid_cb is not None and b == mid_b:
                    mid_cb()
            while pend:
                emit_pv(*pend.pop())

            # normalize into the interleaved stage layout:
            # stage half m = k//2, row slice r in [64*(k%2), 64*(k%2)+64)
            # column s (=8r+j) of this chunk -> stg[m][p, j, r]
            m = 0 if k < 3 else 1
            r0 = 64 * k if k < 3 else 0
            dens, recs, bcrs = [], [], []
            for h in range(HPC):
                den = work.tile([1, 512], F32, tag=f"den{h}", name=f"den{k}_{h}")
                nc.vector.tensor_copy(out=den[:], in_=ot[h][HD : HD + 1, :])
                dens.append(den)
            for h in range(HPC):
                rec = work.tile([1, 512], F32, tag=f"rec{h}", name=f"rec{k}_{h}")
                nc.vector.reciprocal_approx_fast(out=rec[:], in_=dens[h][:])
                recs.append(rec)
            for h in range(HPC):
                bcr = work.tile([HD, 512], F32, tag=f"bcr{h}", name=f"bcr{k}_{h}")
                nc.gpsimd.partition_broadcast(bcr[:], recs[h][:])
                bcrs.append(bcr)
            for h in range(HPC):
                dst = stg[m][
                    64 * h : 64 * (h + 1), :, r0 : r0 + 64
                ].rearrange("p j r -> p r j")
                nc.vector.tensor_tensor(
                    out=dst, in0=ot[h][0:HD, :], in1=bcrs[h][:],
                    op=mybir.AluOpType.mult,
                )
            # ship this chunk's row-slice immediately (64 rows per dest)
            rsl = slice(r0, r0 + 64)
            nc.sync.dma_start(
                out=send_d[m].ap().rearrange("j p r -> p j r")[:, :, rsl],
                in_=stg[m][:, :, rsl],
            )

        def send_half(m):
            nc.gpsimd.collective_compute(
                "AllToAll",
                mybir.AluOpType.bypass,
                replica_groups=[list(range(NC_CORES))],
                ins=[send_d[m].ap().opt()],
                outs=[recv_d[m].ap().opt()],
            )

        at_sb = [None, None]

        def recv_half(m):
            rw = 192 if m == 0 else 64
            at_sb[m] = work.tile([128, NC_CORES, rw], BF16, tag=f"at{m}", name=f"at{m}")
            nc.sync.dma_start(
                out=at_sb[m][:],
                in_=recv_d[m].ap().rearrange("j p w -> p j w"),
            )

        # ---- software-pipelined emission ----
        pq = {}
        pq[0] = proj_chunk(0)
        load_xt(2, nc.gpsimd)
        pq[1] = proj_chunk(1)
        rope_chunk(0, *pq.pop(0))
        load_xt(3, nc.gpsimd)
        attention_chunk(0)
        # wo load now: on the gpsimd queue behind xT c2/c3, off the
        # startup critical path
        for i in range(NDC):
            nc.gpsimd.dma_start(out=wo_sb[:, i, :], in_=wo_e[:, i, :])
        pq[2] = proj_chunk(2)
        rope_chunk(1, *pq.pop(1))
        attention_chunk(1)
        pq[3] = proj_chunk(3)
        rope_chunk(2, *pq.pop(2))
        attention_chunk(2)
        send_half(0)
        recv_half(0)

        def oproj_piece(m, lo, hi, obase, tag):
            # out rows [obase, obase+(hi-lo)) from at part m rows [lo, hi)
            rows = hi - lo
            for dn in range(2):
                op_ps = projp.tile([128, 512], F32, tag="proj", name=f"op{tag}{dn}")
                for j in range(NC_CORES):
                    nc.tensor.matmul(
                        op_ps[0:rows, :],
                        lhsT=at_sb[m][:, j, lo:hi],
                        rhs=wo_sb[:, j, 512 * dn : 512 * (dn + 1)],
                        start=(j == 0),
                        stop=(j == NC_CORES - 1),
                    )
                osb = work.tile([rows, 512], F32, tag=f"osb{tag}{dn}",
                                name=f"osb{tag}{dn}")
                nc.scalar.copy(out=osb[:], in_=op_ps[0:rows, :])
                nc.sync.dma_start(
                    out=out_e[obase : obase + rows, 512 * dn : 512 * (dn + 1)],
                    in_=osb[:],
                )

        rope_chunk(3, *pq.pop(3))
        attention_chunk(3)
        send_half(1)
        recv_half(1)
        # rows 0-191 arrived with AllToAll #1: project them while chunk-3's
        # normalize / send / AllToAll #2 run on DVE, gpsimd and the CC stream
        oproj_piece(0, 0, 128, 0, "a")
        oproj_piece(0, 128, 192, 128, "b")
        oproj_piece(1, 0, 64, 192, "c")

        rope_tmp.__exit__(None, None, None)
        otp_cm.__exit__(None, None, None)
        stp_cm.__exit__(None, None, None)
        ropep_cm.__exit__(None, None, None)
        projp_cm.__exit__(None, None, None)


# ---------------- host side ----------------

_CACHE = {}


def _prep_consts():
    # ppm: lhsT of the signed half-swap M (per 64 block: [[0,-I],[I,0]])
    M = np.zeros((128, 128), np.float32)
    for hb in range(2):
        o = 64 * hb
        for j in range(HALF):
            M[o + j, o + HALF + j] = -1.0
            M[o + HALF + j, o + j] = 1.0
    ppm = M.T.astype(np_bf16)
    idm = np.eye(128, dtype=np_bf16)
    # tri[p, j] = 1 if j >= p (valid: sq >= sk within diagonal block)
    tri = (np.arange(128)[None, :] >= np.arange(128)[:, None]).astype(np_bf16)
    return ppm, idm, tri


def assemble_out(per_core):
    """per_core[c] is core c's [SC, D] slice; rows are round-robin mod 8:
    local row r < 128  -> global row 8r + c        (exchange half 0)
    local row r >= 128 -> global row 1024 + 8(r-128) + c   (half 1)."""
    full = np.empty((S, D), np.float32)
    for c in range(NC_CORES):
        oc = np.asarray(per_core[c], np.float32)
        full[c:1024:8] = oc[0:128]
        full[1024 + c :: 8] = oc[128:256]
    return full.reshape(B, S, D)


def kernel(x, rope_cos, rope_sin, Wq, Wk, Wv, Wo):
    if "nc" not in _CACHE:
        _CACHE["nc"] = build_graph()
    nc = _CACHE["nc"]

    x2 = np.asarray(x, np.float32).reshape(S, D)
    cosT = np.asarray(rope_cos, np.float32).T  # [32, S]
    sinT = np.asarray(rope_sin, np.float32).T
    c2 = np.tile(cosT, (4, 1)).astype(np_bf16)  # [128, S]
    s2 = np.tile(sinT, (4, 1)).astype(np_bf16)
    ppm, idm, tri = _prep_consts()

    Wq = np.asarray(Wq, np.float32)
    Wk = np.asarray(Wk, np.float32)
    Wv = np.asarray(Wv, np.float32)
    Wo = np.asarray(Wo, np.float32)

    def chunked(w):  # [1024, X] -> [128, 8, X] (partition-major d-chunks)
        return np.ascontiguousarray(
            w.reshape(NDC, 128, -1).transpose(1, 0, 2)
        ).astype(np_bf16)

    # xT dram layout [128, NDC, S]: partition p, d-chunk i -> x[:, 128*i + p]
    xT3 = np.ascontiguousarray(
        x2.T.reshape(NDC, 128, NCH, 512).transpose(1, 2, 0, 3)
    ).astype(np_bf16)

    wo_b = chunked(Wo)
    in_maps = []
    for c in range(NC_CORES):
        kv = c // 2
        wq_c = chunked(Wq[:, HPC * HD * c : HPC * HD * (c + 1)])
        wkv_c = chunked(
            np.concatenate(
                [Wk[:, HD * kv : HD * (kv + 1)], Wv[:, HD * kv : HD * (kv + 1)]],
                axis=1,
            )
        )
        in_maps.append(
            {
                "xT": xT3,
                "wq": wq_c,
                "wkv": wkv_c,
                "wo": wo_b,
                "c2": c2,
                "s2": s2,
                "ppm": ppm,
                "idm": idm,
                "tri": tri,
            }
        )

    res = run_bass_kernel_spmd(nc, in_maps, core_ids=list(range(NC_CORES)))
    return assemble_out([res.results[c]["out"] for c in range(NC_CORES)]).astype(
        np.float32
    )


# revision 29
# speedup vs baseline: 1.1257x; 1.1257x over previous
"""Distributed Trainium2 kernel for GQA attention (nn_Attention_76845554860188).

B=1, S=2048, D=1024, NH=16, NKV=4, HD=64, causal, RoPE, 8 NeuronCores.

Sharding: tensor-parallel over heads. Core c owns q-heads {2c, 2c+1} and their
shared GQA kv-head c//2. Each core projects Q/K/V for all 2048 positions and
runs causal flash-style attention for its 2 heads. The attention outputs are
exchanged with two AllToAlls (seq rows assigned round-robin mod 8, global row
= 8*r + c, so every exchange carries rows for every core); each core applies
the full output projection to its 256 assigned rows. Host reassembles the
row interleave (assemble_out).

Performance structure:
  * seq-chunk software pipeline: projection / RoPE / attention emitted per
    512-column chunk; per-chunk SBUF tiles so dependency tracking stays tight.
  * scores for the two heads run concurrently on the PE via row-tiling
    (K=64 each): krot duplicated at partitions 64..127 -> tile_position (64,0).
  * flash ordering: score block b+1 streams while ACT exps block b; PV(b)
    follows. No S^2 probability matrix is materialized.
  * PSUM->SBUF copies and causal masks run on DVE, broadcasts/memsets on
    gpsimd, keeping ACT free for the exp stream (the attention-phase
    co-bottleneck); the normalize chain is head-interleaved and each chunk's
    stage slice is DMA'd to the send buffer as soon as it is normalized.
  * asymmetric exchange: AllToAll #1 carries chunks 0-2 (rows 0-191) and
    fires after chunk 2; its 192 output rows are projected (pieces a+b)
    while chunk-3's normalize/send/AllToAll #2 run on DVE/gpsimd/CC. Only
    the 128KB AllToAll #2 plus a 64-row O-proj piece remain serial.
  * PE warm-up matmuls (on the first-arriving weight tile) ramp HAM during
    the xT DMA fill; xT chunk 0 is the first large DMA issued, per-chunk
    contiguous in DRAM.
"""

import sys

sys.path.insert(0, "/opt/trn_rl_repo")

import numpy as np
import ml_dtypes

import concourse.bass as bass
import concourse.mybir as mybir
import concourse.tile as tile
from concourse import bacc
from concourse.bass_utils import run_bass_kernel_spmd

BF16 = mybir.dt.bfloat16
F32 = mybir.dt.float32

B, S, D = 1, 2048, 1024
NH, NKV, HD = 16, 4, 64
NC_CORES = 8
HPC = NH // NC_CORES  # q heads per core = 2
SC = S // NC_CORES  # seq rows per core = 256
NDC = D // 128  # d chunks = 8
NSB = S // 128  # 128-wide seq blocks = 16
NCH = S // 512  # 512-wide seq chunks = 4
HALF = HD // 2  # 32

np_bf16 = ml_dtypes.bfloat16


def build_graph(taps=False):
    nc = bacc.Bacc(
        "TRN2", target_bir_lowering=False, debug=False, num_devices=NC_CORES
    )

    xT_e = nc.dram_tensor("xT", [128, NCH, NDC, 512], BF16, kind="ExternalInput")
    wq_e = nc.dram_tensor("wq", [128, NDC, HPC * HD], BF16, kind="ExternalInput")
    wkv_e = nc.dram_tensor("wkv", [128, NDC, 2 * HD], BF16, kind="ExternalInput")
    wo_e = nc.dram_tensor("wo", [128, NDC, D], BF16, kind="ExternalInput")
    c2_e = nc.dram_tensor("c2", [128, S], BF16, kind="ExternalInput")
    s2_e = nc.dram_tensor("s2", [128, S], BF16, kind="ExternalInput")
    ppm_e = nc.dram_tensor("ppm", [128, 128], BF16, kind="ExternalInput")
    idm_e = nc.dram_tensor("idm", [128, 128], BF16, kind="ExternalInput")
    tri_e = nc.dram_tensor("tri", [128, 128], BF16, kind="ExternalInput")
    out_e = nc.dram_tensor("out", [SC, D], F32, kind="ExternalOutput")

    send_d = [
        nc.dram_tensor("a2a_send0", [NC_CORES, 128, 192], BF16),
        nc.dram_tensor("a2a_send1", [NC_CORES, 128, 64], BF16),
    ]
    recv_d = [
        nc.dram_tensor("a2a_recv0", [NC_CORES, 128, 192], BF16),
        nc.dram_tensor("a2a_recv1", [NC_CORES, 128, 64], BF16),
    ]
    wup_s = nc.dram_tensor("wup_s", [NC_CORES, 1, 8], BF16)
    wup_r = nc.dram_tensor("wup_r", [NC_CORES, 1, 8], BF16)

    with tile.TileContext(nc) as tc:
        _body(nc, tc, xT_e, wq_e, wkv_e, wo_e, c2_e, s2_e, ppm_e, idm_e, tri_e,
              out_e, send_d, recv_d, wup_s, wup_r)

    nc.compile()
    return nc


def _body(nc, tc, xT_e, wq_e, wkv_e, wo_e, c2_e, s2_e, ppm_e, idm_e, tri_e,
          out_e, send_d, recv_d, wup_s, wup_r):
    from contextlib import ExitStack

    ctx = ExitStack()
    with ctx:
        consts = ctx.enter_context(tc.tile_pool(name="consts", bufs=1))
        work = ctx.enter_context(tc.tile_pool(name="work", bufs=1))
        ptp = ctx.enter_context(tc.tile_pool(name="pt", bufs=6))
        # PSUM: 8 banks total -> proj x2 + rope x1 + st x3 + ot x2
        projp_cm = tc.tile_pool(name="projp", bufs=2, space="PSUM")
        projp = projp_cm.__enter__()
        ropep_cm = tc.tile_pool(name="ropep", bufs=1, space="PSUM")
        ropep = ropep_cm.__enter__()
        stp_cm = tc.tile_pool(name="stp", bufs=3, space="PSUM")
        stp = stp_cm.__enter__()
        otp_cm = tc.tile_pool(name="otp", bufs=2, space="PSUM")
        otp = otp_cm.__enter__()

        # ---- warmup collective, first in collective program order ----
        wup_sb = consts.tile([NC_CORES, 1, 8], BF16, tag="wup")
        nc.gpsimd.memset(wup_sb[:], 0.0)
        nc.scalar.dma_start(out=wup_s.ap(), in_=wup_sb[:])
        nc.gpsimd.collective_compute(
            "AllToAll",
            mybir.AluOpType.bypass,
            replica_groups=[list(range(NC_CORES))],
            ins=[wup_s.ap().opt()],
            outs=[wup_r.ap().opt()],
        )

        # ---- DMA plan ----
        # sync queue:   xT c0, xT c1        (the critical-path inputs)
        # gpsimd queue: xT c2, xT c3, wo    (wo issued later, see below)
        # scalar queue: wq, wkv, consts, c2, s2 (small, never blocks xT)
        xT_sb = consts.tile([128, NDC, S], BF16, tag="xT")

        def load_xt(n, eng):
            sl = slice(512 * n, 512 * (n + 1))
            eng.dma_start(out=xT_sb[:, :, sl], in_=xT_e[:, n])

        load_xt(0, nc.sync)
        load_xt(1, nc.sync)
        wq_sb = consts.tile([128, NDC, HPC * HD], BF16, tag="wq")
        nc.scalar.dma_start(out=wq_sb[:], in_=wq_e.ap())
        wkv_sb = consts.tile([128, NDC, 2 * HD], BF16, tag="wkv")
        nc.scalar.dma_start(out=wkv_sb[:], in_=wkv_e.ap())
        # per-chunk rope-table tiles: chunk 0 lands first so rope c0 isn't
        # gated on the full 1MB of cos/sin; separate tiles keep the
        # dependency tracking per-chunk
        c2t = [consts.tile([128, 512], BF16, tag=f"c2_{n}", name=f"c2_{n}")
               for n in range(NCH)]
        s2t = [consts.tile([128, 512], BF16, tag=f"s2_{n}", name=f"s2_{n}")
               for n in range(NCH)]
        nc.scalar.dma_start(out=c2t[0][:], in_=c2_e[:, 0:512])
        nc.scalar.dma_start(out=s2t[0][:], in_=s2_e[:, 0:512])
        idm_sb = consts.tile([128, 128], BF16, tag="idm")
        nc.scalar.dma_start(out=idm_sb[:], in_=idm_e[:, :])
        ppm_sb = consts.tile([128, 128], BF16, tag="ppm")
        nc.scalar.dma_start(out=ppm_sb[:], in_=ppm_e[:, :])
        tri_sb = consts.tile([128, 128], BF16, tag="tri")
        nc.scalar.dma_start(out=tri_sb[:], in_=tri_e[:, :])
        for n in range(1, NCH):
            nc.scalar.dma_start(out=c2t[n][:], in_=c2_e[:, 512 * n : 512 * (n + 1)])
            nc.scalar.dma_start(out=s2t[n][:], in_=s2_e[:, 512 * n : 512 * (n + 1)])
        wo_sb = consts.tile([128, NDC, D], BF16, tag="wo")

        # ---- PE warm-up on the first weight tile (dep: wq DMA only) ----
        warm_ps = projp.tile([128, 512], F32, tag="proj")
        for r in range(16):
            nc.tensor.matmul(
                warm_ps[:, 0:128],
                lhsT=wq_sb[:, 0, :],
                rhs=wq_sb[:, 0, :],
                start=True,
                stop=True,
            )

        # ---- per-chunk persistent SBUF tiles (tight dependency tracking) ----
        qrot = [work.tile([128, 512], BF16, tag=f"qrot{n}", name=f"qrot{n}") for n in range(NCH)]
        krot2 = [work.tile([128, 512], BF16, tag=f"krot2{n}", name=f"krot2{n}") for n in range(NCH)]
        vTt = [work.tile([64, 512], BF16, tag=f"vT{n}", name=f"vT{n}") for n in range(NCH)]
        vext = [work.tile([128, 4, HD + 1], BF16, tag=f"vext{n}", name=f"vext{n}") for n in range(NCH)]
        for n in range(NCH):
            nc.gpsimd.memset(vext[n][:, :, HD : HD + 1], 1.0)
        # stage buffers per exchange half: [p, dest j, row r] (col s = 8r + j)
        stg = [
            work.tile([128, NC_CORES, 192], BF16, tag="stg0", name="stg0"),
            work.tile([128, NC_CORES, 64], BF16, tag="stg1", name="stg1"),
        ]

        scale = 1.0 / np.sqrt(HD)

        def proj_chunk(n):
            sl = slice(512 * n, 512 * (n + 1))
            q_ps = projp.tile([128, 512], F32, tag="proj", name=f"q{n}")
            for i in range(NDC):
                nc.tensor.matmul(
                    q_ps[:], lhsT=wq_sb[:, i, :], rhs=xT_sb[:, i, sl],
                    start=(i == 0), stop=(i == NDC - 1),
                )
            kv_ps = projp.tile([128, 512], F32, tag="proj", name=f"kv{n}")
            for i in range(NDC):
                nc.tensor.matmul(
                    kv_ps[:], lhsT=wkv_sb[:, i, :], rhs=xT_sb[:, i, sl],
                    start=(i == 0), stop=(i == NDC - 1),
                )
            return q_ps, kv_ps

        rope_tmp = tc.tile_pool(name="ropet", bufs=2)
        ropet = rope_tmp.__enter__()

        def rope_chunk(n, q_ps, kv_ps):
            sl = slice(512 * n, 512 * (n + 1))
            qc = ropet.tile([128, 512], BF16, tag="qc", name=f"qc{n}")
            nc.vector.tensor_tensor(
                out=qc[:], in0=q_ps[:], in1=c2t[n][:], op=mybir.AluOpType.mult
            )
            qs = ropet.tile([128, 512], BF16, tag="qs", name=f"qs{n}")
            nc.vector.tensor_tensor(
                out=qs[:], in0=q_ps[:], in1=s2t[n][:], op=mybir.AluOpType.mult
            )
            qrot_ps = ropep.tile([128, 512], F32, tag="rope", name=f"qr{n}")
            nc.tensor.matmul(
                qrot_ps[:], lhsT=ppm_sb[:], rhs=qs[:], start=True, stop=False
            )
            nc.tensor.matmul(
                qrot_ps[:], lhsT=idm_sb[:], rhs=qc[:], start=False, stop=True
            )
            nc.vector.tensor_copy(out=qrot[n][:], in_=qrot_ps[:])

            kc = ropet.tile([64, 512], BF16, tag="kc", name=f"kc{n}")
            nc.vector.tensor_tensor(
                out=kc[:], in0=kv_ps[0:64, :], in1=c2t[n][0:64, :],
                op=mybir.AluOpType.mult,
            )
            ks = ropet.tile([64, 512], BF16, tag="ks", name=f"ks{n}")
            nc.vector.tensor_tensor(
                out=ks[:], in0=kv_ps[0:64, :], in1=s2t[n][0:64, :],
                op=mybir.AluOpType.mult,
            )
            nc.vector.tensor_copy(out=vTt[n][:], in_=kv_ps[64:128, :])
            krot_ps = ropep.tile([64, 512], F32, tag="rope", name=f"kr{n}")
            nc.tensor.matmul(
                krot_ps[:], lhsT=ppm_sb[0:64, 0:64], rhs=ks[:], start=True, stop=False
            )
            nc.tensor.matmul(
                krot_ps[:], lhsT=idm_sb[0:64, 0:64], rhs=kc[:], start=False, stop=True
            )
            # duplicate krot to partitions 64..127 so head-1 score matmuls
            # run on PE row-groups 2-3 concurrently with head 0
            nc.vector.tensor_copy(out=krot2[n][0:64, :], in_=krot_ps[:])
            nc.vector.tensor_copy(out=krot2[n][64:128, :], in_=krot_ps[:])

            vt_ps = stp.tile([128, 4, HD], BF16, tag="st", name=f"vt{n}")
            for j in range(4):
                nc.tensor.transpose(
                    vt_ps[:, j, :],
                    vTt[n][:, 128 * j : 128 * (j + 1)],
                    idm_sb[0:64, 0:64],
                )
            nc.vector.tensor_copy(out=vext[n][:, :, 0:HD], in_=vt_ps[:])

        def attention_chunk(k, mid_cb=None, mid_b=None):
            nblk = 4 * k + 4
            ot = [
                otp.tile([HD + 1, 512], F32, tag="ot", name=f"ot{h}_{k}")
                for h in range(HPC)
            ]
            pend = []

            def emit_pv(b, pt, qoff, w):
                for h in range(HPC):
                    nc.tensor.matmul(
                        ot[h][:, qoff : qoff + w],
                        lhsT=vext[b // 4][:, b % 4, :],
                        rhs=pt[h][:, 0:w],
                        start=(b == 0),
                        stop=(b == nblk - 1),
                    )

            for b in range(nblk):
                qoff = max(0, 128 * b - 512 * k)
                w = 512 - qoff
                q0 = qoff  # column offset within chunk k's qrot tile
                st0 = stp.tile([128, 512], F32, tag="st", name=f"st0_{k}_{b}")
                st1 = stp.tile([128, 512], F32, tag="st", name=f"st1_{k}_{b}")
                kb = slice(128 * (b % 4), 128 * (b % 4) + 128)
                kt = krot2[b // 4]
                nc.tensor.matmul(
                    st0[:, 0:w], lhsT=kt[0:64, kb],
                    rhs=qrot[k][0:64, q0 : q0 + w], start=True, stop=True,
                )
                nc.tensor.matmul(
                    st1[:, 0:w], lhsT=kt[64:128, kb],
                    rhs=qrot[k][64:128, q0 : q0 + w], start=True, stop=True,
                )
                if pend:
                    emit_pv(*pend.pop())
                pt = [
                    ptp.tile([128, 512], BF16, tag="pt", name=f"pt{h}_{k}_{b}")
                    for h in range(HPC)
                ]
                for h, st in ((0, st0), (1, st1)):
                    nc.scalar.activation(
                        out=pt[h][:, 0:w], in_=st[:, 0:w],
                        func=mybir.ActivationFunctionType.Exp, scale=scale,
                    )
                if 128 * b >= 512 * k:
                    # diagonal block: mask its first 128 cols (kpos > q -> 0)
                    for h in range(HPC):
                        nc.vector.tensor_tensor(
                            out=pt[h][:, 0:128], in0=pt[h][:, 0:128],
                            in1=tri_sb[:], op=mybir.AluOpType.mult,
                        )
                pend.append((b, pt, qoff, w))
                if mid_cb is not None and b == mid_b:
                    mid_cb()
            while pend:
                emit_pv(*pend.pop())

            # normalize into the interleaved stage layout:
            # stage half m = k//2, row slice r in [64*(k%2), 64*(k%2)+64)
            # column s (=8r+j) of this chunk -> stg[m][p, j, r]
            m = 0 if k < 3 else 1
            r0 = 64 * k if k < 3 else 0
            dens, recs, bcrs = [], [], []
            for h in range(HPC):
                den = work.tile([1, 512], F32, tag=f"den{h}", name=f"den{k}_{h}")
                nc.vector.tensor_copy(out=den[:], in_=ot[h][HD : HD + 1, :])
                dens.append(den)
            for h in range(HPC):
                rec = work.tile([1, 512], F32, tag=f"rec{h}", name=f"rec{k}_{h}")
                nc.vector.reciprocal_approx_fast(out=rec[:], in_=dens[h][:])
                recs.append(rec)
            for h in range(HPC):
                bcr = work.tile([HD, 512], F32, tag=f"bcr{h}", name=f"bcr{k}_{h}")
                nc.gpsimd.partition_broadcast(bcr[:], recs[h][:])
                bcrs.append(bcr)
            for h in range(HPC):
                dst = stg[m][
                    64 * h : 64 * (h + 1), :, r0 : r0 + 64
                ].rearrange("p j r -> p r j")
                nc.vector.tensor_tensor(
                    out=dst, in0=ot[h][0:HD, :], in1=bcrs[h][:],
                    op=mybir.AluOpType.mult,
                )
            # ship this chunk's row-slice immediately (64 rows per dest)
            rsl = slice(r0, r0 + 64)
            nc.sync.dma_start(
                out=send_d[m].ap().rearrange("j p r -> p j r")[:, :, rsl],
                in_=stg[m][:, :, rsl],
            )

        def send_half(m):
            nc.gpsimd.collective_compute(
                "AllToAll",
                mybir.AluOpType.bypass,
                replica_groups=[list(range(NC_CORES))],
                ins=[send_d[m].ap().opt()],
                outs=[recv_d[m].ap().opt()],
            )

        at_sb = [None, None]

        def recv_half(m):
            rw = 192 if m == 0 else 64
            at_sb[m] = work.tile([128, NC_CORES, rw], BF16, tag=f"at{m}", name=f"at{m}")
            nc.sync.dma_start(
                out=at_sb[m][:],
                in_=recv_d[m].ap().rearrange("j p w -> p j w"),
            )

        # ---- software-pipelined emission ----
        pq = {}
        pq[0] = proj_chunk(0)
        load_xt(2, nc.gpsimd)
        pq[1] = proj_chunk(1)
        rope_chunk(0, *pq.pop(0))
        load_xt(3, nc.gpsimd)
        attention_chunk(0)
        # wo load now: on the gpsimd queue behind xT c2/c3, off the
        # startup critical path
        for i in range(NDC):
            nc.gpsimd.dma_start(out=wo_sb[:, i, :], in_=wo_e[:, i, :])
        pq[2] = proj_chunk(2)
        rope_chunk(1, *pq.pop(1))
        attention_chunk(1)
        pq[3] = proj_chunk(3)
        rope_chunk(2, *pq.pop(2))
        attention_chunk(2)
        send_half(0)
        recv_half(0)

        def oproj_piece(m, lo, hi, obase, tag):
            # out rows [obase, obase+(hi-lo)) from at part m rows [lo, hi)
            rows = hi - lo
            for dn in range(2):
                op_ps = projp.tile([128, 512], F32, tag="proj", name=f"op{tag}{dn}")
                for j in range(NC_CORES):
                    nc.tensor.matmul(
                        op_ps[0:rows, :],
                        lhsT=at_sb[m][:, j, lo:hi],
                        rhs=wo_sb[:, j, 512 * dn : 512 * (dn + 1)],
                        start=(j == 0),
                        stop=(j == NC_CORES - 1),
                    )
                osb = work.tile([rows, 512], F32, tag=f"osb{tag}{dn}",
                                name=f"osb{tag}{dn}")
                nc.scalar.copy(out=osb[:], in_=op_ps[0:rows, :])
                nc.sync.dma_start(
                    out=out_e[obase : obase + rows, 512 * dn : 512 * (dn + 1)],
                    in_=osb[:],
                )

        rope_chunk(3, *pq.pop(3))
        attention_chunk(3)
        send_half(1)
        recv_half(1)
        # rows 0-191 arrived with AllToAll #1: project them while chunk-3's
        # normalize / send / AllToAll #2 run on DVE, gpsimd and the CC stream
        oproj_piece(0, 0, 128, 0, "a")
        oproj_piece(0, 128, 192, 128, "b")
        oproj_piece(1, 0, 64, 192, "c")

        rope_tmp.__exit__(None, None, None)
        otp_cm.__exit__(None, None, None)
        stp_cm.__exit__(None, None, None)
        ropep_cm.__exit__(None, None, None)
        projp_cm.__exit__(None, None, None)


# ---------------- host side ----------------

_CACHE = {}


def _prep_consts():
    # ppm: lhsT of the signed half-swap M (per 64 block: [[0,-I],[I,0]])
    M = np.zeros((128, 128), np.float32)
    for hb in range(2):
        o = 64 * hb
        for j in range(HALF):
            M[o + j, o + HALF + j] = -1.0
            M[o + HALF + j, o + j] = 1.0
    ppm = M.T.astype(np_bf16)
    idm = np.eye(128, dtype=np_bf16)
    # tri[p, j] = 1 if j >= p (valid: sq >= sk within diagonal block)
    tri = (np.arange(128)[None, :] >= np.arange(128)[:, None]).astype(np_bf16)
    return ppm, idm, tri


def assemble_out(per_core):
    """per_core[c] is core c's [SC, D] slice; rows are round-robin mod 8:
    local row r < 128  -> global row 8r + c        (exchange half 0)
    local row r >= 128 -> global row 1024 + 8(r-128) + c   (half 1)."""
    full = np.empty((S, D), np.float32)
    for c in range(NC_CORES):
        oc = np.asarray(per_core[c], np.float32)
        full[c:1024:8] = oc[0:128]
        full[1024 + c :: 8] = oc[128:256]
    return full.reshape(B, S, D)


def kernel(x, rope_cos, rope_sin, Wq, Wk, Wv, Wo):
    if "nc" not in _CACHE:
        _CACHE["nc"] = build_graph()
    nc = _CACHE["nc"]

    x2 = np.asarray(x, np.float32).reshape(S, D)
    cosT = np.asarray(rope_cos, np.float32).T  # [32, S]
    sinT = np.asarray(rope_sin, np.float32).T
    c2 = np.tile(cosT, (4, 1)).astype(np_bf16)  # [128, S]
    s2 = np.tile(sinT, (4, 1)).astype(np_bf16)
    ppm, idm, tri = _prep_consts()

    Wq = np.asarray(Wq, np.float32)
    Wk = np.asarray(Wk, np.float32)
    Wv = np.asarray(Wv, np.float32)
    Wo = np.asarray(Wo, np.float32)

    def chunked(w):  # [1024, X] -> [128, 8, X] (partition-major d-chunks)
        return np.ascontiguousarray(
            w.reshape(NDC, 128, -1).transpose(1, 0, 2)
        ).astype(np_bf16)

    # xT dram layout [128, NDC, S]: partition p, d-chunk i -> x[:, 128*i + p]
    xT3 = np.ascontiguousarray(
        x2.T.reshape(NDC, 128, NCH, 512).transpose(1, 2, 0, 3)
    ).astype(np_bf16)

    wo_b = chunked(Wo)
    in_maps = []
    for c in range(NC_CORES):
        kv = c // 2
        wq_c = chunked(Wq[:, HPC * HD * c : HPC * HD * (c + 1)])
        wkv_c = chunked(
            np.concatenate(
                [Wk[:, HD * kv : HD * (kv + 1)], Wv[:, HD * kv : HD * (kv + 1)]],
                axis=1,
            )
        )
        in_maps.append(
            {
                "xT": xT3,
                "wq": wq_c,
                "wkv": wkv_c,
                "wo": wo_b,
                "c2": c2,
                "s2": s2,
                "ppm": ppm,
                "idm": idm,
                "tri": tri,
            }
        )

    res = run_bass_kernel_spmd(nc, in_maps, core_ids=list(range(NC_CORES)))
    return assemble_out([res.results[c]["out"] for c in range(NC_CORES)]).astype(
        np.float32
    )
